# revision 25
# baseline (speedup 1.0000x reference)
"""Multi-head causal attention (B=4, S=2048, DM=1024, H=16) on 8 TRN2 cores.

Sharding: core = 2*b + hg  (b = batch 0..3, hg = head-group 0..1, 8 heads each).
Each core computes, for its batch and its 8 heads:
    Q^T = (Wq_hg)^T x^T, K^T likewise   [512, 2048]  (d-major)
    V   = x Wv_hg                       [2048, 512]  (s-major, per-head 65-col
                                                      blocks with a ones column
                                                      for the softmax row-sums;
                                                      plus an fp8 copy in
                                                      80-col aligned blocks)
    S^T(kt) = K_kt Q^T / masked-exp -> A^T tiles  [128k, q]
        off-diagonal (fully causal-valid) tiles are stored as fp8 pairs;
        the diagonal chunk stays bf16 (also keeps few-key early rows at
        bf16 precision)
    O^T_unnorm[65, q]: off-diagonal key tiles via fp8 DoubleRow matmuls
        (2 key tiles contracted per pass), diagonal via bf16 matmuls;
        row 64 = row-sums; normalized via reciprocal on the PSUM row-sum
        row + GpSimd partition_broadcast (no DRAM round trips);
        result -> O^T [512, 2048] bf16
    out_part = O^T-contracted projection with Wo rows of this head group,
        written bf16.  proj(st-1) is interleaved after attn(st) so the
        projection fills stage-boundary PE bubbles and the output DMAs
        spread across the whole timeline.
Host sums the two head-group partials per batch (fp32) and adds bo.

exp uses a -2.0 offset so the unnormalized fp8e4m3 softmax weights stay
well inside [0, 240] (max causal logit ~6); the e^-2 factor cancels in the
row-sum normalization. Host transposes x, pre-tiles all operands into
contiguous [128, kt, cols] blocks (2KB+ DMA lines), and converts dtypes,
so no on-device transposes are needed anywhere.
"""

import sys

for _p in ("/opt/trn_rl_repo",):
    if _p not in sys.path:
        sys.path.insert(0, _p)

import numpy as np
import ml_dtypes

import concourse.bass as bass
import concourse.mybir as mybir
import concourse.tile as tile
from concourse import bacc
from concourse.bass_utils import run_bass_kernel_spmd


def _pbcast(row_ap, nparts):
    """Broadcast a [1, N] DRAM AP along the partition axis -> [nparts, N]."""
    return bass.AP(
        tensor=row_ap.tensor,
        offset=row_ap.offset,
        ap=[[0, nparts]] + list(row_ap.ap)[1:],
    )


def _reshape128(row_ap, width):
    """View a [1, 128*width] DRAM AP as [128, width]."""
    return bass.AP(
        tensor=row_ap.tensor,
        offset=row_ap.offset,
        ap=[[width, 128], [1, width]],
    )


B, S, DM, H, DK = 4, 2048, 1024, 16, 64
HPC = 8          # heads per core
DQK = 512        # q/k/v width per core
NPAIR = 4        # head pairs per core (one per 128-row d-tile)
SC = S // 512    # 512-wide q/s chunks
KT = S // 128    # 128-wide key tiles
A_BUFS = 6       # live bf16 A^T tiles (diagonal chunk only: 4 + pipeline)

BF16 = mybir.dt.bfloat16
F32 = mybir.dt.float32
F8 = mybir.dt.float8e4
BF = ml_dtypes.bfloat16
EXP = mybir.ActivationFunctionType.Exp
MUL = mybir.AluOpType.mult
DR = mybir.MatmulPerfMode.DoubleRow
EXPOFF = 2.0

LAST_RESULT = None


def _emit(nc, nkt, schedule="pipelined"):
    """Emit the whole per-core kernel. nkt = contraction tiles (8, or 9 when
    biases are folded in via an augmented ones-row in xT)."""
    # host pre-tiles everything: partition-major contiguous blocks
    xT = nc.dram_tensor("xT", [SC, 128, nkt, 512], BF16, kind="ExternalInput").ap()
    wq = nc.dram_tensor("wq", [128, nkt, DQK], BF16, kind="ExternalInput").ap()
    wk = nc.dram_tensor("wk", [128, nkt, DQK], BF16, kind="ExternalInput").ap()
    wv = nc.dram_tensor("wv", [128, nkt, DQK], BF16, kind="ExternalInput").ap()
    wo = nc.dram_tensor("wo", [128, NPAIR, DM], BF16, kind="ExternalInput").ap()
    msk = nc.dram_tensor("mask", [128, 128], BF16, kind="ExternalInput").ap()
    out = nc.dram_tensor("out", [S, DM], BF16, kind="ExternalOutput").ap()

    with tile.TileContext(nc) as tc:
        with (
            tc.tile_pool(name="big", bufs=1) as big,
            tc.tile_pool(name="wqkv", bufs=1) as wp,
            tc.tile_pool(name="xc", bufs=3) as xcp,
            tc.tile_pool(name="ap", bufs=A_BUFS) as apool,
            tc.tile_pool(name="a8", bufs=8) as a8p,
            tc.tile_pool(name="rcp", bufs=5) as rcpp,
            tc.tile_pool(name="bc", bufs=4) as bcp,
            tc.tile_pool(name="ob", bufs=3) as obp,
            tc.tile_pool(name="mm", bufs=2, space="PSUM") as mmp,
            tc.tile_pool(name="sc", bufs=2, space="PSUM") as scp,
            tc.tile_pool(name="otp", bufs=2, space="PSUM") as otpp,
            tc.tile_pool(name="rb", bufs=8, space="DRAM") as rbp,
        ):
            # Q^T in two variants with the other pair-half's rows zeroed, so
            # scores matmuls use full K=128 contraction at partition base 0
            # (concurrent disjoint-row-group matmuls fault on this setup).
            qt_e = big.tile([128, NPAIR, S], BF16, tag="qte")
            qt_o = big.tile([128, NPAIR, S], BF16, tag="qto")
            kt_ = big.tile([128, NPAIR, S], BF16, tag="kt")
            v = big.tile([128, KT, HPC * 65], BF16, tag="v")
            ot = big.tile([128, NPAIR, S], BF16, tag="ot")
            wo_sb = big.tile([128, NPAIR, DM], BF16, tag="wo")
            mask_sb = big.tile([128, 128], BF16, tag="mask")
            v8 = big.tile([128, KT, HPC * 80], F8, tag="v8")
            expoff_sb = big.tile([128, 1], F32, tag="expoff")

            # zero fills: only the never-written halves of qt_e/qt_o need
            # zeroing (~7us each at ~1 elem/cycle); split across GpSimd and
            # the (until-first-cast idle) Vector queue so both finish before
            # the first Q-chain PSUM copy-outs
            nc.gpsimd.memset(qt_e[64:128, :, :], 0.0)
            nc.vector.memset(qt_o[0:64, :, :], 0.0)
            nc.gpsimd.memset(expoff_sb, -EXPOFF)
            v_ones = v.rearrange("p kt (h c) -> p kt h c", c=65)[:, :, :, 64:65]
            nc.gpsimd.memset(v_ones, 1.0)
            # fp8 V blocks are 80 wide (vs 65): dual-fp8 weight loads need
            # 16-aligned column offsets/counts; col 64 is the ones column,
            # cols 65-79 are zero padding
            v8_pad = v8.rearrange("p kt (h c) -> p kt h c", c=80)[:, :, :, 64:80]
            nc.gpsimd.memset(v8_pad, 0.0)
            v8_ones = v8.rearrange("p kt (h c) -> p kt h c", c=80)[:, :, :, 64:65]
            nc.gpsimd.memset(v8_ones, 1.0)

            wq_sb = wp.tile([128, nkt, DQK], BF16, tag="wq")
            wk_sb = wp.tile([128, nkt, DQK], BF16, tag="wk")
            wv_sb = wp.tile([128, nkt, DQK], BF16, tag="wv")

            def qkv_dma(st, eng=None):
                eng = eng or nc.sync
                xc = xcp.tile([128, nkt, 512], BF16, tag="xc")
                eng.dma_start(out=xc, in_=xT[st])
                return xc

            # stage-0 operands land first, split in halves so the first
            # matmul chain starts after two ~0.5MB transfers (per-k-tile
            # splitting costs ~1us of queue overhead per DMA and paces the
            # chains instead); wk behind wq on the scalar queue (exp starts
            # much later), bulk wv/mask/wo behind xc0 on sync
            kh = nkt // 2
            xc0 = xcp.tile([128, nkt, 512], BF16, tag="xc")
            nc.scalar.dma_start(out=wq_sb[:, 0:kh, :], in_=wq[:, 0:kh, :])
            nc.sync.dma_start(out=xc0[:, 0:kh, :], in_=xT[0, :, 0:kh, :])
            nc.scalar.dma_start(out=wq_sb[:, kh:nkt, :], in_=wq[:, kh:nkt, :])
            nc.sync.dma_start(out=xc0[:, kh:nkt, :], in_=xT[0, :, kh:nkt, :])
            nc.scalar.dma_start(out=wk_sb, in_=wk)
            nc.sync.dma_start(out=wv_sb, in_=wv)
            nc.sync.dma_start(out=mask_sb, in_=msk)
            xc1 = qkv_dma(1)
            nc.sync.dma_start(out=wo_sb, in_=wo)

            def qkv_units(st, xc):
                """The 12 projection chains of stage st as separate units."""
                ss = slice(st * 512, (st + 1) * 512)
                units = []

                def qk_chain(w_sb, dst, dt_i):
                    ps = mmp.tile([128, 512], F32, tag="mm")
                    for k in range(nkt):
                        nc.tensor.matmul(
                            out=ps,
                            lhsT=w_sb[:, k, dt_i * 128 : (dt_i + 1) * 128],
                            rhs=xc[:, k, :],
                            start=(k == 0),
                            stop=(k == nkt - 1),
                        )
                    if dst is None:
                        nc.vector.tensor_copy(
                            out=qt_e[0:64, dt_i, ss], in_=ps[0:64, :]
                        )
                        nc.vector.tensor_copy(
                            out=qt_o[64:128, dt_i, ss], in_=ps[64:128, :]
                        )
                    else:
                        nc.vector.tensor_copy(out=dst[:, dt_i, ss], in_=ps)

                def v_chain(ssub):
                    ps = mmp.tile([128, 512], F32, tag="mm")
                    for k in range(nkt):
                        nc.tensor.matmul(
                            out=ps,
                            lhsT=xc[:, k, ssub * 128 : (ssub + 1) * 128],
                            rhs=wv_sb[:, k, :],
                            start=(k == 0),
                            stop=(k == nkt - 1),
                        )
                    kti = st * 4 + ssub
                    ps4 = ps.rearrange("p (h d) -> p h d", d=DK)
                    v4 = v[:, kti, :].rearrange("p (h c) -> p h c", c=65)
                    nc.vector.tensor_copy(out=v4[:, :, 0:DK], in_=ps4)
                    v84 = v8[:, kti, :].rearrange("p (h c) -> p h c", c=80)
                    nc.vector.tensor_copy(out=v84[:, :, 0:DK], in_=ps4)

                for w_sb, dst in ((wq_sb, None), (wk_sb, kt_)):
                    for dt_i in range(NPAIR):
                        units.append(
                            lambda w=w_sb, d=dst, i=dt_i: qk_chain(w, d, i)
                        )
                for ssub in range(4):
                    units.append(lambda s=ssub: v_chain(s))
                return units

            def attn_units(pr, c):
                """Attention for head pair pr, q chunk c, as a list of
                units: one per score tile, plus the two AV/normalize
                blocks.  Off-diagonal (fully causal-valid) key tiles store
                A in fp8 pairs and contract with fp8 V via DoubleRow
                matmuls (2 key tiles per pass); the diagonal chunk stays
                bf16, which also keeps the few-key early rows at bf16
                precision."""
                a_tiles = {}
                a8_tiles = {}
                units = []

                def scores(kti):
                    cs0 = kti // 4
                    qoff = 128 * (kti % 4) if c == cs0 else 0
                    ps = scp.tile([128, 2, 512], F32, tag="sc")
                    for hh in range(2):
                        qsrc = qt_e if hh == 0 else qt_o
                        nc.tensor.matmul(
                            out=ps[:, hh, qoff:512],
                            lhsT=kt_[:, pr, kti * 128 : (kti + 1) * 128],
                            rhs=qsrc[:, pr, c * 512 + qoff : (c + 1) * 512],
                            start=True,
                            stop=True,
                        )
                    if c == cs0:
                        at = apool.tile([128, 2, 512], BF16, tag="a")
                        nc.scalar.activation(
                            out=at[:, :, qoff:512],
                            in_=ps[:, :, qoff:512],
                            func=EXP,
                            scale=0.125,
                            bias=expoff_sb,
                        )
                        dg = at[:, :, qoff : qoff + 128]
                        # stays on Vector: the Pool (Q7) runs this
                        # multiply ~5x slower than the DVE, which
                        # lengthens the diagonal-AV gating chain more
                        # than the queueing it avoids
                        nc.vector.tensor_tensor(
                            out=dg, in0=dg, in1=_pbcast2(mask_sb), op=MUL
                        )
                        a_tiles[(kti, c)] = at
                    else:
                        key = (kti // 2, c)
                        if key not in a8_tiles:
                            a8t = a8p.tile([128, 2, 2, 512], F8, tag="a8")
                            a8_tiles[key] = a8t
                        nc.scalar.activation(
                            out=a8_tiles[key][:, kti % 2, :, :],
                            in_=ps,
                            func=EXP,
                            scale=0.125,
                            bias=expoff_sb,
                        )

                def av_norm(hh):
                    # hh=1 first: its normalization ends with an extra
                    # SBUF-shift DMA, so putting it first keeps that
                    # DMA off the exposed tail of the last stage
                    C = c
                    h = pr * 2 + hh
                    ops = otpp.tile([128, 512], F32, tag="otp")
                    for t2 in range(2 * C):
                        nc.tensor.matmul(
                            out=ops[0:80, :],
                            lhsT=v8[
                                :, 2 * t2 : 2 * t2 + 2,
                                h * 80 : (h + 1) * 80,
                            ],
                            rhs=a8_tiles[(t2, C)][:, :, hh, :],
                            start=(t2 == 0),
                            stop=False,
                            perf_mode=DR,
                        )
                    for j in range(4):
                        k2 = 4 * C + j
                        qoff = 128 * j
                        nc.tensor.matmul(
                            out=ops[0:65, qoff:512],
                            lhsT=v[:, k2, h * 65 : (h + 1) * 65],
                            rhs=a_tiles[(k2, C)][:, hh, qoff:512],
                            start=(C == 0 and j == 0),
                            stop=(j == 3),
                        )
                    # normalize: reciprocal on a DMA-reshaped [128, 4]
                    # view + DMA partition broadcast, all hops on the
                    # (unloaded) sync queue.  The chain latency is hidden
                    # everywhere: mid-kernel because proj consumes ot a
                    # full stage later, and for the last-emitted pair
                    # because proj(SC-2)'s units are held back to run
                    # right after the final AV block.
                    csl = slice(C * 512, (C + 1) * 512)
                    rs = rcpp.tile([128, 512], F32, tag="rs")
                    nc.vector.tensor_copy(
                        out=rs[0:65, :], in_=ops[0:65, :]
                    )
                    rd = rbp.tile([1, 512], F32, tag="rb")
                    nc.sync.dma_start(out=rd, in_=rs[64:65, :])
                    r2 = rcpp.tile([128, 4], F32, tag="r2")
                    nc.sync.dma_start(out=r2, in_=_reshape128(rd, 4))
                    r2b = rcpp.tile([128, 4], F32, tag="r2b")
                    nc.vector.reciprocal(out=r2b, in_=r2)
                    rd2 = rbp.tile([1, 512], F32, tag="rb2")
                    nc.sync.dma_start(out=_reshape128(rd2, 4), in_=r2b)
                    bc = bcp.tile([128, 512], F32, tag="bcw")
                    nc.sync.dma_start(
                        out=bc[0:64, :], in_=_pbcast(rd2, 64)
                    )
                    bcv = bc[0:64, :]
                    if hh == 0:
                        nc.vector.tensor_tensor(
                            out=ot[0:64, pr, csl],
                            in0=rs[0:64, :],
                            in1=bcv,
                            op=MUL,
                        )
                    else:
                        tmp = bcp.tile([64, 512], BF16, tag="tmp")
                        nc.vector.tensor_tensor(
                            out=tmp, in0=rs[0:64, :], in1=bcv, op=MUL
                        )
                        nc.sync.dma_start(out=ot[64:128, pr, csl], in_=tmp)

                for kti in range(4 * c + 4):
                    units.append(lambda k=kti: scores(k))
                units.append(lambda: av_norm(1))
                units.append(lambda: av_norm(0))
                return units

            def proj_units(chunk):
                def unit(ssub):
                    srow = chunk * 4 + ssub
                    ob = obp.tile([128, 2, 512], BF16, tag="ob")
                    for mc in range(2):
                        ps = mmp.tile([128, 512], F32, tag="mm")
                        for dt_i in range(NPAIR):
                            nc.tensor.matmul(
                                out=ps,
                                lhsT=ot[:, dt_i, srow * 128 : (srow + 1) * 128],
                                rhs=wo_sb[:, dt_i, mc * 512 : (mc + 1) * 512],
                                start=(dt_i == 0),
                                stop=(dt_i == NPAIR - 1),
                            )
                        nc.vector.tensor_copy(out=ob[:, mc, :], in_=ps)
                    # single [128, 1024] write: 2KB contiguous lines
                    nc.sync.dma_start(
                        out=out[srow * 128 : (srow + 1) * 128, :],
                        in_=ob,
                    )
                return [lambda s=ssub: unit(s) for ssub in range(4)]

            def emit_interleaved(A, B):
                """Emit attention units (A, exp-generating) with B units
                (qkv/proj chains, pure PE) spread proportionally between
                them, so the Scalar exp stream overlaps matmul-dense work
                instead of pacing the PE."""
                n, m = len(A), len(B)
                if n == 0:
                    for b in B:
                        b()
                    return
                bi = 0
                for i, a in enumerate(A):
                    while bi < m and bi * n <= i * m:
                        B[bi]()
                        bi += 1
                    a()
                while bi < m:
                    B[bi]()
                    bi += 1

            if schedule == "pipelined":
                # software-pipelined: attention for chunk st-1 runs
                # interleaved with the qkv projections of stage st (its
                # operands were finished in stage st-1), so the Scalar
                # exp stream overlaps the PE-dense projection chains.
                # proj(0..1) spread through the qkv-free epilogue with
                # attn(3); the epilogue emits pair 3 first and holds
                # proj(2) back so it covers the last pair's (pr=2)
                # normalization-chain latency before proj(3).
                xcs = [xc0, xc1] + [None] * (SC - 2)
                for st in range(SC + 1):
                    if st + 2 < SC:
                        xcs[st + 2] = qkv_dma(st + 2)
                    epi = st == SC
                    A = []
                    if st >= 1:
                        for pr in ((3, 0, 1, 2) if epi else range(NPAIR)):
                            A += attn_units(pr, st - 1)
                    B = qkv_units(st, xcs[st]) if not epi else []
                    if epi:
                        B = proj_units(0) + proj_units(1)
                    emit_interleaved(A, B)
                for u in proj_units(SC - 2) + proj_units(SC - 1):
                    u()
            else:
                raise ValueError(schedule)
    return nc


def _pbcast2(mask_sb):
    """mask [128, 128] -> [128, 2, 128] with broadcast middle dim."""
    m = mask_sb[:, :]
    ap = list(m.ap)
    return bass.AP(tensor=m.tensor, offset=m.offset, ap=[ap[0], [0, 2], ap[1]])


_NC_CACHE = {}


def _get_nc(nkt):
    if nkt not in _NC_CACHE:
        nc = bacc.Bacc(
            "TRN2",
            target_bir_lowering=False,
            debug=False,
            enable_asserts=False,
            num_devices=8,
        )
        _emit(nc, nkt)
        nc.compile()
        _NC_CACHE[nkt] = nc
    return _NC_CACHE[nkt]


def kernel(**inputs):
    x = np.asarray(inputs["x"], dtype=np.float32)
    mask = np.asarray(inputs["mask"]).reshape(S, S)
    Wq = np.asarray(inputs["Wq"], dtype=np.float32)
    bq = np.asarray(inputs["bq"], dtype=np.float32)
    Wk = np.asarray(inputs["Wk"], dtype=np.float32)
    bk = np.asarray(inputs["bk"], dtype=np.float32)
    Wv = np.asarray(inputs["Wv"], dtype=np.float32)
    bv = np.asarray(inputs["bv"], dtype=np.float32)
    Wo = np.asarray(inputs["Wo"], dtype=np.float32)
    bo = np.asarray(inputs["bo"], dtype=np.float32)

    assert np.array_equal(
        mask, np.tril(np.ones((S, S), dtype=bool))
    ), "kernel specialized for the causal (tril) mask"

    bias_zero = not (bq.any() or bk.any() or bv.any())
    nkt = 8 if bias_zero else 9
    nc = _get_nc(nkt)

    # local diag-block mask in (k, q) layout: valid when q >= k
    mtile = np.triu(np.ones((128, 128), dtype=np.float32)).astype(BF)

    def tile_w(w, b):
        """[DM(+pad), 512] -> partition-major [128, nkt, 512] contiguous."""
        if not bias_zero:
            pad = np.zeros((128, w.shape[1]), dtype=np.float32)
            pad[0] = b
            w = np.vstack([w, pad])
        return np.ascontiguousarray(
            w.reshape(nkt, 128, w.shape[1]).transpose(1, 0, 2)
        ).astype(BF)

    in_maps = []
    for core in range(8):
        b, hg = divmod(core, 2)
        cols = slice(hg * DQK, (hg + 1) * DQK)
        xT = x[b].T
        if not bias_zero:
            pad = np.zeros((128, S), dtype=np.float32)
            pad[0] = 1.0
            xT = np.vstack([xT, pad])
        # [nkt*128, S] -> chunk-major [SC, 128, nkt, 512]
        xTt = np.ascontiguousarray(
            xT.reshape(nkt, 128, SC, 512).transpose(2, 1, 0, 3)
        ).astype(BF)
        wo_t = np.ascontiguousarray(
            Wo[cols, :].reshape(NPAIR, 128, DM).transpose(1, 0, 2)
        ).astype(BF)
        in_maps.append(
            {
                "xT": xTt,
                "wq": tile_w(Wq[:, cols], bq[cols]),
                "wk": tile_w(Wk[:, cols], bk[cols]),
                "wv": tile_w(Wv[:, cols], bv[cols]),
                "wo": wo_t,
                "mask": mtile,
            }
        )

    res = run_bass_kernel_spmd(nc, in_maps, core_ids=list(range(8)))
    global LAST_RESULT
    LAST_RESULT = res
    parts = [np.asarray(r["out"], dtype=np.float32) for r in res.results]
    out = np.stack(
        [parts[2 * b_] + parts[2 * b_ + 1] for b_ in range(B)]
    ) + bo.astype(np.float32)
    return out.astype(np.float32)


# revision 28
# speedup vs baseline: 1.0200x; 1.0200x over previous
"""Multi-head causal attention (B=4, S=2048, DM=1024, H=16) on 8 TRN2 cores.

Sharding: core = 2*b + hg  (b = batch 0..3, hg = head-group 0..1, 8 heads each).
Each core computes, for its batch and its 8 heads:
    Q^T = (Wq_hg)^T x^T, K^T likewise   [512, 2048]  (d-major)
    V   = x Wv_hg                       [2048, 512]  (s-major, per-head 65-col
                                                      blocks with a ones column
                                                      for the softmax row-sums;
                                                      plus an fp8 copy in
                                                      80-col aligned blocks)
    S^T(kt) = K_kt Q^T / masked-exp -> A^T tiles  [128k, q]
        off-diagonal (fully causal-valid) tiles are stored as fp8 pairs;
        the diagonal chunk stays bf16 (also keeps few-key early rows at
        bf16 precision)
    O^T_unnorm[65, q]: off-diagonal key tiles via fp8 DoubleRow matmuls
        (2 key tiles contracted per pass), diagonal via bf16 matmuls;
        row 64 = row-sums; normalized via reciprocal on the PSUM row-sum
        row + GpSimd partition_broadcast (no DRAM round trips);
        result -> O^T [512, 2048] bf16
    out_part = O^T-contracted projection with Wo rows of this head group,
        written bf16.  proj(st-1) is interleaved after attn(st) so the
        projection fills stage-boundary PE bubbles and the output DMAs
        spread across the whole timeline.
Host sums the two head-group partials per batch (fp32) and adds bo.

exp uses a -2.0 offset so the unnormalized fp8e4m3 softmax weights stay
well inside [0, 240] (max causal logit ~6); the e^-2 factor cancels in the
row-sum normalization. Host transposes x, pre-tiles all operands into
contiguous [128, kt, cols] blocks (2KB+ DMA lines), and converts dtypes,
so no on-device transposes are needed anywhere.
"""

import sys

for _p in ("/opt/trn_rl_repo",):
    if _p not in sys.path:
        sys.path.insert(0, _p)

import numpy as np
import ml_dtypes

import concourse.bass as bass
import concourse.mybir as mybir
import concourse.tile as tile
from concourse import bacc
from concourse.bass_utils import run_bass_kernel_spmd


def _pbcast(row_ap, nparts):
    """Broadcast a [1, N] DRAM AP along the partition axis -> [nparts, N]."""
    return bass.AP(
        tensor=row_ap.tensor,
        offset=row_ap.offset,
        ap=[[0, nparts]] + list(row_ap.ap)[1:],
    )


def _reshape128(row_ap, width):
    """View a [1, 128*width] DRAM AP as [128, width]."""
    return bass.AP(
        tensor=row_ap.tensor,
        offset=row_ap.offset,
        ap=[[width, 128], [1, width]],
    )


B, S, DM, H, DK = 4, 2048, 1024, 16, 64
HPC = 8          # heads per core
DQK = 512        # q/k/v width per core
NPAIR = 4        # head pairs per core (one per 128-row d-tile)
SC = S // 512    # 512-wide q/s chunks
KT = S // 128    # 128-wide key tiles
A_BUFS = 6       # live bf16 A^T tiles (diagonal chunk only: 4 + pipeline)

BF16 = mybir.dt.bfloat16
F32 = mybir.dt.float32
F8 = mybir.dt.float8e4
BF = ml_dtypes.bfloat16
EXP = mybir.ActivationFunctionType.Exp
MUL = mybir.AluOpType.mult
DR = mybir.MatmulPerfMode.DoubleRow
EXPOFF = 2.0

LAST_RESULT = None


def _emit(nc, nkt, schedule="pipelined"):
    """Emit the whole per-core kernel. nkt = contraction tiles (8, or 9 when
    biases are folded in via an augmented ones-row in xT)."""
    # host pre-tiles everything: partition-major contiguous blocks
    xT = nc.dram_tensor("xT", [SC, 128, nkt, 512], BF16, kind="ExternalInput").ap()
    wq = nc.dram_tensor("wq", [128, nkt, DQK], BF16, kind="ExternalInput").ap()
    wk = nc.dram_tensor("wk", [128, nkt, DQK], BF16, kind="ExternalInput").ap()
    wv = nc.dram_tensor("wv", [128, nkt, DQK], BF16, kind="ExternalInput").ap()
    wo = nc.dram_tensor("wo", [128, NPAIR, DM], BF16, kind="ExternalInput").ap()
    msk = nc.dram_tensor("mask", [128, 128], BF16, kind="ExternalInput").ap()
    out = nc.dram_tensor("out", [S, DM], BF16, kind="ExternalOutput").ap()

    with tile.TileContext(nc) as tc:
        with (
            tc.tile_pool(name="big", bufs=1) as big,
            tc.tile_pool(name="wqkv", bufs=1) as wp,
            tc.tile_pool(name="xc", bufs=3) as xcp,
            tc.tile_pool(name="ap", bufs=A_BUFS) as apool,
            tc.tile_pool(name="a8", bufs=8) as a8p,
            tc.tile_pool(name="rcp", bufs=4) as rcpp,
            tc.tile_pool(name="rcq", bufs=2) as rcpq,
            tc.tile_pool(name="bc", bufs=3) as bcp,
            tc.tile_pool(name="ob", bufs=3) as obp,
            tc.tile_pool(name="mm", bufs=2, space="PSUM") as mmp,
            tc.tile_pool(name="sc", bufs=2, space="PSUM") as scp,
            tc.tile_pool(name="otp", bufs=2, space="PSUM") as otpp,
            tc.tile_pool(name="rb", bufs=8, space="DRAM") as rbp,
        ):
            # Q^T in two variants with the other pair-half's rows zeroed, so
            # scores matmuls use full K=128 contraction at partition base 0
            # (concurrent disjoint-row-group matmuls fault on this setup).
            qt_e = big.tile([128, NPAIR, S], BF16, tag="qte")
            qt_o = big.tile([128, NPAIR, S], BF16, tag="qto")
            kt_ = big.tile([128, NPAIR, S], BF16, tag="kt")
            v = big.tile([128, KT, HPC * 65], BF16, tag="v")
            ot = big.tile([128, NPAIR, S], BF16, tag="ot")
            wo_sb = big.tile([128, NPAIR, DM], BF16, tag="wo")
            mask_sb = big.tile([128, 128], BF16, tag="mask")
            v8 = big.tile([128, KT, HPC * 80], F8, tag="v8")
            expoff_sb = big.tile([128, 1], F32, tag="expoff")

            # zero fills: only the never-written halves of qt_e/qt_o need
            # zeroing (~7us each at ~1 elem/cycle); split across GpSimd and
            # the (until-first-cast idle) Vector queue so both finish before
            # the first Q-chain PSUM copy-outs
            nc.gpsimd.memset(qt_e[64:128, :, :], 0.0)
            nc.vector.memset(qt_o[0:64, :, :], 0.0)
            nc.gpsimd.memset(expoff_sb, -EXPOFF)
            v_ones = v.rearrange("p kt (h c) -> p kt h c", c=65)[:, :, :, 64:65]
            nc.gpsimd.memset(v_ones, 1.0)
            # fp8 V blocks are 80 wide (vs 65): dual-fp8 weight loads need
            # 16-aligned column offsets/counts; col 64 is the ones column,
            # cols 65-79 are zero padding
            v8_pad = v8.rearrange("p kt (h c) -> p kt h c", c=80)[:, :, :, 64:80]
            nc.gpsimd.memset(v8_pad, 0.0)
            v8_ones = v8.rearrange("p kt (h c) -> p kt h c", c=80)[:, :, :, 64:65]
            nc.gpsimd.memset(v8_ones, 1.0)

            wq_sb = wp.tile([128, nkt, DQK], BF16, tag="wq")
            wk_sb = wp.tile([128, nkt, DQK], BF16, tag="wk")
            wv_sb = wp.tile([128, nkt, DQK], BF16, tag="wv")

            def qkv_dma(st, eng=None):
                eng = eng or nc.sync
                xc = xcp.tile([128, nkt, 512], BF16, tag="xc")
                eng.dma_start(out=xc, in_=xT[st])
                return xc

            # stage-0 operands land first, split in halves so the first
            # matmul chain starts after two ~0.5MB transfers (per-k-tile
            # splitting costs ~1us of queue overhead per DMA and paces the
            # chains instead); wk behind wq on the scalar queue (exp starts
            # much later), bulk wv/mask/wo behind xc0 on sync
            kh = nkt // 2
            xc0 = xcp.tile([128, nkt, 512], BF16, tag="xc")
            nc.scalar.dma_start(out=wq_sb[:, 0:kh, :], in_=wq[:, 0:kh, :])
            nc.sync.dma_start(out=xc0[:, 0:kh, :], in_=xT[0, :, 0:kh, :])
            nc.scalar.dma_start(out=wq_sb[:, kh:nkt, :], in_=wq[:, kh:nkt, :])
            nc.sync.dma_start(out=xc0[:, kh:nkt, :], in_=xT[0, :, kh:nkt, :])
            nc.scalar.dma_start(out=wk_sb, in_=wk)
            nc.sync.dma_start(out=wv_sb, in_=wv)
            nc.sync.dma_start(out=mask_sb, in_=msk)
            xc1 = qkv_dma(1)
            nc.sync.dma_start(out=wo_sb, in_=wo)

            def qkv_units(st, xc):
                """The 12 projection chains of stage st as separate units."""
                ss = slice(st * 512, (st + 1) * 512)
                units = []

                def qk_chain(w_sb, dst, dt_i):
                    ps = mmp.tile([128, 512], F32, tag="mm")
                    for k in range(nkt):
                        nc.tensor.matmul(
                            out=ps,
                            lhsT=w_sb[:, k, dt_i * 128 : (dt_i + 1) * 128],
                            rhs=xc[:, k, :],
                            start=(k == 0),
                            stop=(k == nkt - 1),
                        )
                    if dst is None:
                        nc.vector.tensor_copy(
                            out=qt_e[0:64, dt_i, ss], in_=ps[0:64, :]
                        )
                        nc.vector.tensor_copy(
                            out=qt_o[64:128, dt_i, ss], in_=ps[64:128, :]
                        )
                    else:
                        nc.vector.tensor_copy(out=dst[:, dt_i, ss], in_=ps)

                def v_chain(ssub):
                    ps = mmp.tile([128, 512], F32, tag="mm")
                    for k in range(nkt):
                        nc.tensor.matmul(
                            out=ps,
                            lhsT=xc[:, k, ssub * 128 : (ssub + 1) * 128],
                            rhs=wv_sb[:, k, :],
                            start=(k == 0),
                            stop=(k == nkt - 1),
                        )
                    kti = st * 4 + ssub
                    ps4 = ps.rearrange("p (h d) -> p h d", d=DK)
                    v4 = v[:, kti, :].rearrange("p (h c) -> p h c", c=65)
                    nc.vector.tensor_copy(out=v4[:, :, 0:DK], in_=ps4)
                    v84 = v8[:, kti, :].rearrange("p (h c) -> p h c", c=80)
                    nc.vector.tensor_copy(out=v84[:, :, 0:DK], in_=ps4)

                for w_sb, dst in ((wq_sb, None), (wk_sb, kt_)):
                    for dt_i in range(NPAIR):
                        units.append(
                            lambda w=w_sb, d=dst, i=dt_i: qk_chain(w, d, i)
                        )
                for ssub in range(4):
                    units.append(lambda s=ssub: v_chain(s))
                return units

            def attn_units(pr, c):
                """Attention for head pair pr, q chunk c, as a list of
                units: one per score tile, plus the two AV/normalize
                blocks.  Off-diagonal (fully causal-valid) key tiles store
                A in fp8 pairs and contract with fp8 V via DoubleRow
                matmuls (2 key tiles per pass); the diagonal chunk stays
                bf16, which also keeps the few-key early rows at bf16
                precision."""
                a_tiles = {}
                a8_tiles = {}
                units = []

                def scores(kti):
                    cs0 = kti // 4
                    qoff = 128 * (kti % 4) if c == cs0 else 0
                    ps = scp.tile([128, 2, 512], F32, tag="sc")
                    for hh in range(2):
                        qsrc = qt_e if hh == 0 else qt_o
                        nc.tensor.matmul(
                            out=ps[:, hh, qoff:512],
                            lhsT=kt_[:, pr, kti * 128 : (kti + 1) * 128],
                            rhs=qsrc[:, pr, c * 512 + qoff : (c + 1) * 512],
                            start=True,
                            stop=True,
                        )
                    if c == cs0:
                        at = apool.tile([128, 2, 512], BF16, tag="a")
                        nc.scalar.activation(
                            out=at[:, :, qoff:512],
                            in_=ps[:, :, qoff:512],
                            func=EXP,
                            scale=0.125,
                            bias=expoff_sb,
                        )
                        dg = at[:, :, qoff : qoff + 128]
                        # stays on Vector: the Pool (Q7) runs this
                        # multiply ~5x slower than the DVE, which
                        # lengthens the diagonal-AV gating chain more
                        # than the queueing it avoids
                        nc.vector.tensor_tensor(
                            out=dg, in0=dg, in1=_pbcast2(mask_sb), op=MUL
                        )
                        a_tiles[(kti, c)] = at
                    else:
                        key = (kti // 2, c)
                        if key not in a8_tiles:
                            a8t = a8p.tile([128, 2, 2, 512], F8, tag="a8")
                            a8_tiles[key] = a8t
                        nc.scalar.activation(
                            out=a8_tiles[key][:, kti % 2, :, :],
                            in_=ps,
                            func=EXP,
                            scale=0.125,
                            bias=expoff_sb,
                        )

                def av_norm(hh):
                    # hh=1 first: its normalization ends with an extra
                    # SBUF-shift DMA, so putting it first keeps that
                    # DMA off the exposed tail of the last stage
                    C = c
                    h = pr * 2 + hh
                    ops = otpp.tile([128, 512], F32, tag="otp")
                    for t2 in range(2 * C):
                        nc.tensor.matmul(
                            out=ops[0:80, :],
                            lhsT=v8[
                                :, 2 * t2 : 2 * t2 + 2,
                                h * 80 : (h + 1) * 80,
                            ],
                            rhs=a8_tiles[(t2, C)][:, :, hh, :],
                            start=(t2 == 0),
                            stop=False,
                            perf_mode=DR,
                        )
                    for j in range(4):
                        k2 = 4 * C + j
                        qoff = 128 * j
                        nc.tensor.matmul(
                            out=ops[0:65, qoff:512],
                            lhsT=v[:, k2, h * 65 : (h + 1) * 65],
                            rhs=a_tiles[(k2, C)][:, hh, qoff:512],
                            start=(C == 0 and j == 0),
                            stop=(j == 3),
                        )
                    # normalize.  Mid-kernel chains: reciprocal on a
                    # DMA-reshaped [128, 4] view + DMA partition
                    # broadcast, all hops on the (unloaded) sync queue --
                    # the latency hides because proj consumes ot a full
                    # stage later.  The epilogue's last-emitted pair
                    # instead uses the direct [1, 512]-lane reciprocal
                    # (3.3us on the then-idle DVE) + Pool partition
                    # broadcast, so no DMA-completion latency sits on the
                    # exposed tail before the final projections.
                    csl = slice(C * 512, (C + 1) * 512)
                    last = pr == 2 and c == SC - 1
                    if last:
                        rcp = rcpq.tile([1, 512], F32, tag="rcp")
                        nc.vector.reciprocal(out=rcp, in_=ops[64:65, :])
                        bcl = bcp.tile([64, 512], F32, tag="bc")
                        nc.gpsimd.partition_broadcast(
                            out_ap=bcl, in_ap=rcp, channels=64
                        )
                        rs = rcpp.tile([128, 512], F32, tag="rs")
                        nc.vector.tensor_copy(
                            out=rs[0:64, :], in_=ops[0:64, :]
                        )
                        bcv = bcl
                    else:
                        rs = rcpp.tile([128, 512], F32, tag="rs")
                        nc.vector.tensor_copy(
                            out=rs[0:65, :], in_=ops[0:65, :]
                        )
                        rd = rbp.tile([1, 512], F32, tag="rb")
                        nc.sync.dma_start(out=rd, in_=rs[64:65, :])
                        r2 = rcpp.tile([128, 4], F32, tag="r2")
                        nc.sync.dma_start(out=r2, in_=_reshape128(rd, 4))
                        r2b = rcpp.tile([128, 4], F32, tag="r2b")
                        nc.vector.reciprocal(out=r2b, in_=r2)
                        rd2 = rbp.tile([1, 512], F32, tag="rb2")
                        nc.sync.dma_start(out=_reshape128(rd2, 4), in_=r2b)
                        bc = bcp.tile([128, 512], F32, tag="bcw")
                        nc.sync.dma_start(
                            out=bc[0:64, :], in_=_pbcast(rd2, 64)
                        )
                        bcv = bc[0:64, :]
                    if hh == 0:
                        nc.vector.tensor_tensor(
                            out=ot[0:64, pr, csl],
                            in0=rs[0:64, :],
                            in1=bcv,
                            op=MUL,
                        )
                    else:
                        tmp = bcp.tile([64, 512], BF16, tag="tmp")
                        nc.vector.tensor_tensor(
                            out=tmp, in0=rs[0:64, :], in1=bcv, op=MUL
                        )
                        nc.sync.dma_start(out=ot[64:128, pr, csl], in_=tmp)

                for kti in range(4 * c + 4):
                    units.append(lambda k=kti: scores(k))
                units.append(lambda: av_norm(1))
                units.append(lambda: av_norm(0))
                return units

            def proj_units(chunk):
                def unit(ssub):
                    srow = chunk * 4 + ssub
                    ob = obp.tile([128, 2, 512], BF16, tag="ob")
                    for mc in range(2):
                        ps = mmp.tile([128, 512], F32, tag="mm")
                        for dt_i in range(NPAIR):
                            nc.tensor.matmul(
                                out=ps,
                                lhsT=ot[:, dt_i, srow * 128 : (srow + 1) * 128],
                                rhs=wo_sb[:, dt_i, mc * 512 : (mc + 1) * 512],
                                start=(dt_i == 0),
                                stop=(dt_i == NPAIR - 1),
                            )
                        nc.vector.tensor_copy(out=ob[:, mc, :], in_=ps)
                    # single [128, 1024] write: 2KB contiguous lines
                    nc.sync.dma_start(
                        out=out[srow * 128 : (srow + 1) * 128, :],
                        in_=ob,
                    )
                return [lambda s=ssub: unit(s) for ssub in range(4)]

            def emit_interleaved(A, B):
                """Emit attention units (A, exp-generating) with B units
                (qkv/proj chains, pure PE) spread proportionally between
                them, so the Scalar exp stream overlaps matmul-dense work
                instead of pacing the PE."""
                n, m = len(A), len(B)
                if n == 0:
                    for b in B:
                        b()
                    return
                bi = 0
                for i, a in enumerate(A):
                    while bi < m and bi * n <= i * m:
                        B[bi]()
                        bi += 1
                    a()
                while bi < m:
                    B[bi]()
                    bi += 1

            if schedule == "pipelined":
                # software-pipelined: attention for chunk st-1 runs
                # interleaved with the qkv projections of stage st (its
                # operands were finished in stage st-1), so the Scalar
                # exp stream overlaps the PE-dense projection chains.
                # proj(0..1) spread through the qkv-free epilogue with
                # attn(3); the epilogue emits pair 3 first and holds
                # proj(2) back so it covers the last pair's (pr=2)
                # normalization-chain latency before proj(3).
                xcs = [xc0, xc1] + [None] * (SC - 2)
                for st in range(SC):
                    if st + 2 < SC:
                        xcs[st + 2] = qkv_dma(st + 2)
                    A = []
                    if st >= 1:
                        for pr in range(NPAIR):
                            A += attn_units(pr, st - 1)
                    emit_interleaved(A, qkv_units(st, xcs[st]))
                # epilogue: attn(3) with pair 3 first (it needs no filler
                # -- Scalar still lags from stage 3), then the B units
                # weighted over pairs 0..2 where the scp/exp lockstep
                # would otherwise starve the PE; the last 2 proj(2) units
                # cover pair 2's fast normalization chain
                for u in attn_units(3, SC - 1):
                    u()
                A = []
                for pr in (0, 1, 2):
                    A += attn_units(pr, SC - 1)
                Bspread = proj_units(0) + proj_units(1) + proj_units(2)
                emit_interleaved(A, Bspread[:-2])
                for u in Bspread[-2:] + proj_units(SC - 1):
                    u()
            else:
                raise ValueError(schedule)
    return nc


def _pbcast2(mask_sb):
    """mask [128, 128] -> [128, 2, 128] with broadcast middle dim."""
    m = mask_sb[:, :]
    ap = list(m.ap)
    return bass.AP(tensor=m.tensor, offset=m.offset, ap=[ap[0], [0, 2], ap[1]])


_NC_CACHE = {}


def _get_nc(nkt):
    if nkt not in _NC_CACHE:
        nc = bacc.Bacc(
            "TRN2",
            target_bir_lowering=False,
            debug=False,
            enable_asserts=False,
            num_devices=8,
        )
        _emit(nc, nkt)
        nc.compile()
        _NC_CACHE[nkt] = nc
    return _NC_CACHE[nkt]


def kernel(**inputs):
    x = np.asarray(inputs["x"], dtype=np.float32)
    mask = np.asarray(inputs["mask"]).reshape(S, S)
    Wq = np.asarray(inputs["Wq"], dtype=np.float32)
    bq = np.asarray(inputs["bq"], dtype=np.float32)
    Wk = np.asarray(inputs["Wk"], dtype=np.float32)
    bk = np.asarray(inputs["bk"], dtype=np.float32)
    Wv = np.asarray(inputs["Wv"], dtype=np.float32)
    bv = np.asarray(inputs["bv"], dtype=np.float32)
    Wo = np.asarray(inputs["Wo"], dtype=np.float32)
    bo = np.asarray(inputs["bo"], dtype=np.float32)

    assert np.array_equal(
        mask, np.tril(np.ones((S, S), dtype=bool))
    ), "kernel specialized for the causal (tril) mask"

    bias_zero = not (bq.any() or bk.any() or bv.any())
    nkt = 8 if bias_zero else 9
    nc = _get_nc(nkt)

    # local diag-block mask in (k, q) layout: valid when q >= k
    mtile = np.triu(np.ones((128, 128), dtype=np.float32)).astype(BF)

    def tile_w(w, b):
        """[DM(+pad), 512] -> partition-major [128, nkt, 512] contiguous."""
        if not bias_zero:
            pad = np.zeros((128, w.shape[1]), dtype=np.float32)
            pad[0] = b
            w = np.vstack([w, pad])
        return np.ascontiguousarray(
            w.reshape(nkt, 128, w.shape[1]).transpose(1, 0, 2)
        ).astype(BF)

    in_maps = []
    for core in range(8):
        b, hg = divmod(core, 2)
        cols = slice(hg * DQK, (hg + 1) * DQK)
        xT = x[b].T
        if not bias_zero:
            pad = np.zeros((128, S), dtype=np.float32)
            pad[0] = 1.0
            xT = np.vstack([xT, pad])
        # [nkt*128, S] -> chunk-major [SC, 128, nkt, 512]
        xTt = np.ascontiguousarray(
            xT.reshape(nkt, 128, SC, 512).transpose(2, 1, 0, 3)
        ).astype(BF)
        wo_t = np.ascontiguousarray(
            Wo[cols, :].reshape(NPAIR, 128, DM).transpose(1, 0, 2)
        ).astype(BF)
        in_maps.append(
            {
                "xT": xTt,
                "wq": tile_w(Wq[:, cols], bq[cols]),
                "wk": tile_w(Wk[:, cols], bk[cols]),
                "wv": tile_w(Wv[:, cols], bv[cols]),
                "wo": wo_t,
                "mask": mtile,
            }
        )

    res = run_bass_kernel_spmd(nc, in_maps, core_ids=list(range(8)))
    global LAST_RESULT
    LAST_RESULT = res
    parts = [np.asarray(r["out"], dtype=np.float32) for r in res.results]
    out = np.stack(
        [parts[2 * b_] + parts[2 * b_ + 1] for b_ in range(B)]
    ) + bo.astype(np.float32)
    return out.astype(np.float32)
